# revision 7
# baseline (speedup 1.0000x reference)
"""DenseCapsule routing kernel for 8 Trainium2 NeuronCores — factorized.

Problem: x [B=64, I=2048, Din=8], weight [O=64, I=2048, Dout=16, Din=8]
  x_hat = einsum('oidk,bik->boid', w, x); 3 rounds of dynamic routing
  (softmax over O, weighted i-sum, squash, agreement update); out [B, O, Dout].

Strategy: shard I across the 8 cores (IL=256 each). x_hat is NEVER
materialized. Both routing contractions factor through W:
  s[b,o,d]  = sum_{i,k} W[o,i,d,k] * (c[b,o,i] x[b,i,k])   (PE, contract i)
  db[b,o,i] = sum_k x[b,i,k] * u[b,o,i,k],
  u[b,o,i,k]= sum_d W[o,i,d,k] v[b,o,d]                    (PE, contract d)
The only HBM traffic after setup is one [128,1024] f32 AllReduce of the
s-partials per routing iteration.

Index conventions (per core):
  o = 8T + 4t + j,  T in [0,8), t in {0,1}, j in [0,4)
  softmax slot s = 4T + j in [0,32); SBUF partition p = 64t + b%64
  padded (j,u,d) free index 32j + 16u + d (u=1 slots are zero padding)
"""

import sys

sys.path.insert(0, "/opt/trn_rl_repo")

import numpy as np
import ml_dtypes

import concourse.bass as bass
import concourse.tile as tile
from concourse import bacc, mybir
from concourse.bass_utils import run_bass_kernel_spmd

F32 = mybir.dt.float32
BF16 = mybir.dt.bfloat16

B, I, DIN, O, DOUT = 64, 2048, 8, 64, 16
NCORES = 8
IL = I // NCORES          # 256 i's per core
ICH = IL // 128           # 2 i-chunks of 128
EPS = 1e-8
PAD = 128                 # padded (j,u,d) block
SKIP_U = False            # timing variant: skip u_phases
SKIP_YK = False           # timing variant: s-MM rhs = XKT (no cT/yk)
SKIP_AR = False           # timing variant: local DMA instead of AllReduce
SKIP_SMM = False          # timing variant: skip s-MM/evict/transpose in s_phase
SFREE = 512               # packed (T,j,d) free elems of the AR payload


def _squash_padded(nc, pool, sgl, vout, vdtype):
    """vout[p, T, 32j+d] = squash(sgl) over d; padded layout, u=0 slots only.

    sgl: [128, 8, 128] f32 SBUF; vout [128, 8, 128] (u=1 slots untouched).
    """
    sv = sgl[:, :, :, :]  # packed [p, T, j, d]
    n2 = pool.tile([128, 8, 4], F32, tag="n2")
    sq = pool.tile([128, 8, 4, DOUT], F32, tag="sq")
    nc.vector.tensor_tensor(sq, sv, sv, op=mybir.AluOpType.mult)
    nc.vector.tensor_reduce(
        n2, sq, axis=mybir.AxisListType.X, op=mybir.AluOpType.add,
    )
    np1 = pool.tile([128, 8, 4], F32, tag="np1")
    nc.vector.tensor_scalar_add(np1, n2, 1.0)
    r1 = pool.tile([128, 8, 4], F32, tag="r1")
    nc.vector.reciprocal(r1, np1)
    nrm = pool.tile([128, 8, 4], F32, tag="nrm")
    nc.scalar.activation(nrm, n2, mybir.ActivationFunctionType.Sqrt)
    nre = pool.tile([128, 8, 4], F32, tag="nre")
    nc.vector.tensor_scalar_add(nre, nrm, EPS)
    r2 = pool.tile([128, 8, 4], F32, tag="r2")
    nc.vector.reciprocal(r2, nre)
    sc = pool.tile([128, 8, 4], F32, tag="sc")
    nc.vector.tensor_tensor(sc, n2, r1, op=mybir.AluOpType.mult)
    sc2 = pool.tile([128, 8, 4], F32, tag="sc2")
    nc.vector.tensor_tensor(sc2, sc, r2, op=mybir.AluOpType.mult)
    scb = bass.AP(
        tensor=sc2.tensor, offset=sc2.offset,
        ap=[sc2.ap[0], [4, 8], [1, 4], [0, DOUT]],
    )
    vv = bass.AP(
        tensor=vout.tensor, offset=vout.offset,
        ap=[vout.ap[0], [PAD, 8], [32, 4], [1, DOUT]],
    )
    nc.vector.tensor_tensor(vv, sv, scb, op=mybir.AluOpType.mult)


def _squash_packed(nc, pool, sgl, vout):
    """vout (packed [128, 8, 4, 16] f32) = squash(sgl bf16) over d."""
    sv = sgl[:, :, :, :]
    n2 = pool.tile([128, 8, 4], F32, tag="n2")
    sq = pool.tile([128, 8, 4, DOUT], F32, tag="sq")
    nc.vector.tensor_tensor(sq, sv, sv, op=mybir.AluOpType.mult)
    nc.vector.tensor_reduce(
        n2, sq, axis=mybir.AxisListType.X, op=mybir.AluOpType.add,
    )
    np1 = pool.tile([128, 8, 4], F32, tag="np1")
    nc.vector.tensor_scalar_add(np1, n2, 1.0)
    r1 = pool.tile([128, 8, 4], F32, tag="r1")
    nc.vector.reciprocal(r1, np1)
    nrm = pool.tile([128, 8, 4], F32, tag="nrm")
    nc.scalar.activation(nrm, n2, mybir.ActivationFunctionType.Sqrt)
    nre = pool.tile([128, 8, 4], F32, tag="nre")
    nc.vector.tensor_scalar_add(nre, nrm, EPS)
    r2 = pool.tile([128, 8, 4], F32, tag="r2")
    nc.vector.reciprocal(r2, nre)
    sc = pool.tile([128, 8, 4], F32, tag="sc")
    nc.vector.tensor_tensor(sc, n2, r1, op=mybir.AluOpType.mult)
    sc2 = pool.tile([128, 8, 4], F32, tag="sc2")
    nc.vector.tensor_tensor(sc2, sc, r2, op=mybir.AluOpType.mult)
    scb = bass.AP(
        tensor=sc2.tensor, offset=sc2.offset,
        ap=[sc2.ap[0], [4, 8], [1, 4], [0, DOUT]],
    )
    nc.vector.tensor_tensor(vout, sv, scb, op=mybir.AluOpType.mult)


def build():
    nc = bacc.Bacc()
    w2 = nc.declare_dram_parameter("w2", [128, 8, 2, IL * DIN], BF16, isOutput=False)
    wk = nc.declare_dram_parameter("wk", [128, ICH, O, DIN, DOUT], BF16, isOutput=False)
    wf = nc.declare_dram_parameter("wf", [128, 16, O * DOUT], BF16, isOutput=False)
    xik = nc.declare_dram_parameter("xik", [128, 16, B], BF16, isOutput=False)
    x2d = nc.declare_dram_parameter("x2d", [128, IL * DIN], BF16, isOutput=False)
    xkt = nc.declare_dram_parameter("xkt", [128, ICH, DIN, B], BF16, isOutput=False)
    idn = nc.declare_dram_parameter("idn", [128, 128], BF16, isOutput=False)
    out = nc.declare_dram_parameter("out", [B, O, DOUT], F32, isOutput=True)

    groups = [list(range(NCORES))]

    with tile.TileContext(nc) as tc:
        with (
            tc.tile_pool(name="dram", bufs=1, space="DRAM") as dram,
            tc.tile_pool(name="consts", bufs=1) as consts,
            tc.tile_pool(name="persist", bufs=1) as persist,
            tc.tile_pool(name="sqp", bufs=1) as sqp,
        ):
            sp = [dram.tile([128, SFREE], F32, name=f"sp{q}") for q in range(3)]
            sr = [
                dram.tile([128, SFREE], F32, addr_space="Shared", name=f"sr{q}")
                for q in range(3)
            ]

            WF = consts.tile([128, 16, O * DOUT], BF16)
            nc.sync.dma_start(out=WF, in_=wf[:, :, :])
            XIK = consts.tile([128, 16, B], BF16)
            nc.sync.dma_start(out=XIK, in_=xik[:, :, :])
            ID = consts.tile([128, 128], BF16)
            nc.sync.dma_start(out=ID, in_=idn[:, :])
            W2 = consts.tile([128, 8, 2, IL * DIN], BF16)
            X2D = consts.tile([128, IL * DIN], BF16)
            XKT = consts.tile([128, ICH, DIN, B], BF16)
            WK = consts.tile([128, ICH, O, DIN, DOUT], BF16)

            sE = persist.tile([128, 16, B], BF16)     # evicted s-MM slices
            nc.vector.memset(sE, 0.0)
            SGLS = persist.tile([128, 8, 4, DOUT], F32)   # packed s partials
            SGLB = persist.tile([128, 8, 4, DOUT], F32)   # all-reduced s
            V3 = persist.tile([128, 8, 4, DOUT], F32)     # final v (f32)
            VV = persist.tile([128, 8, PAD], BF16)    # v (padded, bf16)
            nc.vector.memset(VV, 0.0)
            V2 = persist.tile([128, 8, 2, B], BF16)   # transposed v for u-MM
            E2 = persist.tile([128, 32, IL], BF16)    # exp(db2)
            EC = persist.tile([128, 32, IL], BF16)    # c2, then e3
            SMS = persist.tile([128, IL], F32)        # softmax sum (both halves)
            SMR = persist.tile([128, IL], BF16)       # softmax recip (bcast)

            def do_ar(q):
                if SKIP_AR:
                    nc.sync.dma_start(out=sr[q][:], in_=sp[q][:])
                else:
                    nc.gpsimd.collective_compute(
                        "AllReduce", mybir.AluOpType.add, replica_groups=groups,
                        ins=[sp[q][:]], outs=[sr[q][:]],
                    )

            def s_phase(q, coeff):
                """s-MMs for all o using rhs slices from coeff(s_slot, ch, t, k)
                -> AllReduce q -> SGL."""
                with (
                    tc.tile_pool(name=f"ps_s{q}", bufs=2, space="PSUM") as pss,
                    tc.tile_pool(name=f"ps_t{q}", bufs=2, space="PSUM") as pst,
                    tc.tile_pool(name=f"ev{q}", bufs=2) as ev,
                ):
                    for T in range(8):
                        if SKIP_SMM:
                            for j in range(4):
                                for ch in range(ICH):
                                    coeff(4 * T + j, ch)
                            continue
                        s8f = [
                            pss.tile([128, 512], F32, tag="s8",
                                     name=f"s8_{q}_{T}_{t}")
                            for t in range(2)
                        ]
                        s8 = [sf[:, 0:B] for sf in s8f]
                        for j in range(4):
                            s_slot = 4 * T + j
                            for ch in range(ICH):
                                rhs_td = coeff(s_slot, ch)
                                for t in range(2):
                                    o = 8 * T + 4 * t + j
                                    for k in range(DIN):
                                        nc.tensor.matmul(
                                            s8f[t][32 * j : 32 * j + 16, 0:B],
                                            WK[:, ch, o, k, :],
                                            rhs_td(t, k),
                                            start=(ch == 0 and k == 0),
                                            stop=(ch == ICH - 1 and k == DIN - 1),
                                            tile_position=(0, 32 * j),
                                        )
                        # evict the 4 real 16-row slices of each half into sE
                        for t in range(2):
                            G = 2 * T + t
                            for j in range(4):
                                nc.scalar.copy(
                                    sE[32 * j : 32 * j + 16, G, :],
                                    s8[t][32 * j : 32 * j + 16, :],
                                )
                        # transpose the (t-pair) into [p=(t,b), (j,u,d)]
                        tpf = pst.tile([128, 1024], BF16, tag="tp",
                                       name=f"tp{q}_{T}")
                        nc.tensor.transpose(
                            tpf[:, 0:128], sE[:, 2 * T : 2 * T + 2, :], ID[:, :]
                        )
                        tpp = bass.AP(  # select u=0: [p, j, d]
                            tensor=tpf.tensor, offset=tpf.offset,
                            ap=[tpf.ap[0], [32, 4], [1, DOUT]],
                        )
                        nc.scalar.copy(SGLS[:, T, :, :], tpp)
                    nc.sync.dma_start(
                        out=sp[q],
                        in_=SGLS.rearrange("p a b c -> p (a b c)"),
                    )
                do_ar(q)
                nc.sync.dma_start(
                    out=SGLB.rearrange("p a b c -> p (a b c)"), in_=sr[q][:]
                )

            def v2_prep():
                """V2[32j+16u+d, T, t, b] = VV transposed per (T, t)."""
                with (
                    tc.tile_pool(name="psv", bufs=2, space="PSUM") as psv,
                ):
                    for T in range(8):
                        for t in range(2):
                            vpf = psv.tile([128, 1024], BF16, tag="vp")
                            nc.tensor.transpose(
                                vpf[:, 0:B],
                                VV[64 * t : 64 * t + 64, T, :],
                                ID[64 * t : 64 * t + 64, 64 * t : 64 * t + 64],
                            )
                            nc.scalar.copy(V2[:, T, t, :], vpf[:, 0:B])

            def u_phase(it):
                """u-MMs + db + exp into E2 (it=1) or EC=E2*exp(db3) (it=2).

                W2/X2D free index is k-major: k*IL + i. The k-sum is a
                unit-stride TT-add tree (2x mode; tensor_reduce is 1x-only).
                """
                with (
                    tc.tile_pool(name=f"psu{it}", bufs=2, space="PSUM") as psu,
                    tc.tile_pool(name=f"ub{it}", bufs=2) as ubp,
                    tc.tile_pool(name=f"tt{it}", bufs=2) as ttp,
                    tc.tile_pool(name=f"db{it}", bufs=3) as dbp,
                ):
                    for T in range(8):
                        for j in range(4):
                            s_slot = 4 * T + j
                            pu = psu.tile(
                                [128, 2048], F32, tag="pu",
                                name=f"pu{it}_{T}_{j}",
                            )
                            for t in range(2):
                                for h in range(4):
                                    nc.tensor.matmul(
                                        pu[64 * t : 64 * t + 64,
                                           512 * h : 512 * h + 512],
                                        V2[32 * j : 32 * j + 16, T, t, :],
                                        W2[32 * j : 32 * j + 16, T, t,
                                           512 * h : 512 * h + 512],
                                        start=True, stop=True,
                                        tile_position=(32 * j, 64 * t),
                                    )
                            ub = ubp.tile([128, 2048], BF16, tag="ub")
                            nc.scalar.copy(ub, pu)
                            tt = ttp.tile([128, DIN, IL], BF16, tag="tt")
                            nc.vector.tensor_tensor(
                                tt,
                                ub.rearrange("p (k i) -> p k i", k=DIN),
                                X2D.rearrange("p (k i) -> p k i", k=DIN),
                                op=mybir.AluOpType.mult,
                            )
                            nc.vector.tensor_tensor(
                                tt[:, 0:4, :], tt[:, 0:4, :], tt[:, 4:8, :],
                                op=mybir.AluOpType.add,
                            )
                            nc.vector.tensor_tensor(
                                tt[:, 0:2, :], tt[:, 0:2, :], tt[:, 2:4, :],
                                op=mybir.AluOpType.add,
                            )
                            db = dbp.tile([128, IL], BF16, tag="db")
                            nc.vector.tensor_tensor(
                                db, tt[:, 0, :], tt[:, 1, :],
                                op=mybir.AluOpType.add,
                            )
                            if it == 1:
                                nc.scalar.activation(
                                    E2[:, s_slot, :], db,
                                    mybir.ActivationFunctionType.Exp,
                                )
                            else:
                                ex = dbp.tile([128, IL], BF16, tag="ex")
                                nc.scalar.activation(
                                    ex, db, mybir.ActivationFunctionType.Exp,
                                )
                                nc.vector.tensor_tensor(
                                    EC[:, s_slot, :],
                                    E2[:, s_slot, :], ex,
                                    op=mybir.AluOpType.mult,
                                )

            def softmax(e_buf, c_buf):
                """c = e / sum_o e ; sum over 32 slots on both halves."""
                with tc.tile_pool(name="smx", bufs=1) as smx:
                    s16 = smx.tile([128, 16, IL], BF16, tag="s16")
                    nc.vector.tensor_tensor(
                        s16, e_buf[:, 0:16, :], e_buf[:, 16:32, :],
                        op=mybir.AluOpType.add,
                    )
                    nc.vector.tensor_tensor(
                        s16[:, 0:8, :], s16[:, 0:8, :], s16[:, 8:16, :],
                        op=mybir.AluOpType.add,
                    )
                    nc.vector.tensor_tensor(
                        s16[:, 0:4, :], s16[:, 0:4, :], s16[:, 4:8, :],
                        op=mybir.AluOpType.add,
                    )
                    nc.vector.tensor_tensor(
                        s16[:, 0:2, :], s16[:, 0:2, :], s16[:, 2:4, :],
                        op=mybir.AluOpType.add,
                    )
                    with nc.allow_low_precision("softmax sum in f32 out"):
                        nc.vector.tensor_tensor(
                            SMS, s16[:, 0, :], s16[:, 1, :],
                            op=mybir.AluOpType.add,
                        )
                    half = smx.tile([64, IL], F32, tag="half")
                    nc.sync.dma_start(out=half, in_=SMS[64:128, :])
                    tot = smx.tile([64, IL], F32, tag="tot")
                    nc.vector.tensor_tensor(
                        tot, SMS[0:64, :], half, op=mybir.AluOpType.add,
                    )
                    rec = smx.tile([64, IL], F32, tag="rec")
                    nc.vector.reciprocal(rec, tot)
                    nc.vector.tensor_copy(SMR[0:64, :], rec)
                    nc.sync.dma_start(out=SMR[64:128, :], in_=SMR[0:64, :])
                    rb = bass.AP(
                        tensor=SMR.tensor, offset=SMR.offset,
                        ap=[SMR.ap[0], [0, 32], [1, IL]],
                    )
                    nc.vector.tensor_tensor(
                        c_buf, e_buf, rb, op=mybir.AluOpType.mult,
                    )

            # ---------------- iteration 1: uniform c, one big matmul ----
            with (
                tc.tile_pool(name="ps1", bufs=1, space="PSUM") as ps1,
                tc.tile_pool(name="s1e", bufs=1) as s1e,
            ):
                p1 = ps1.tile([B, O * DOUT], F32, tag="p1")
                for c in range(16):
                    for h in range(2):
                        nc.tensor.matmul(
                            p1[:, 512 * h : 512 * h + 512],
                            XIK[:, c, :],
                            WF[:, c, 512 * h : 512 * h + 512],
                            start=(c == 0), stop=(c == 15),
                        )
                # evict * (1/O) with on-the-fly reorder to packed layout:
                # s1p[b, t, 64T+16j+d] = p1[b, (8T+4t+j)*16+d] / O
                s1p = s1e.tile([B, 2, SFREE], F32, tag="s1p")
                for t in range(2):
                    pin = bass.AP(
                        tensor=p1.tensor,
                        offset=p1.offset + (4 * t) * DOUT,
                        ap=[p1.ap[0], [8 * DOUT, 8], [DOUT, 4], [1, DOUT]],
                    )
                    nc.scalar.mul(s1p[:, t, :].rearrange(
                        "b (T jd) -> b T jd", T=8), pin, 1.0 / O)
                for t in range(2):
                    dst = bass.AP(
                        tensor=sp[0].tensor,
                        offset=sp[0].offset + 64 * t * SFREE,
                        ap=[[SFREE, B], [1, SFREE]],
                    )
                    nc.sync.dma_start(out=dst, in_=s1p[:, t, :])
            nc.sync.dma_start(out=W2, in_=w2[:, :, :, :])
            nc.sync.dma_start(out=X2D, in_=x2d[:, :])
            nc.sync.dma_start(out=XKT, in_=xkt[:, :, :, :])
            nc.sync.dma_start(out=WK, in_=wk[:, :, :, :, :])
            do_ar(0)
            nc.sync.dma_start(
                out=SGLB.rearrange("p a b c -> p (a b c)"), in_=sr[0][:]
            )
            _squash_padded(nc, sqp, SGLB, VV, BF16)
            v2_prep()

            # ---------------- iteration 2 ----------------
            if SKIP_U:
                nc.vector.memset(E2, 1.0)
            else:
                u_phase(1)
            softmax(E2, EC)

            with (
                tc.tile_pool(name="psc2", bufs=3, space="PSUM") as psc,
                tc.tile_pool(name="ct2", bufs=3) as ctp,
                tc.tile_pool(name="yk2", bufs=3) as ykp,
            ):
                def coeff2(s_slot, ch):
                    if SKIP_YK:
                        return lambda t, k: XKT[:, ch, k, :]
                    cpf = psc.tile([128, 1024], BF16, tag="cp",
                                   name=f"cp2_{s_slot}_{ch}")
                    nc.tensor.transpose(
                        cpf[:, 0:128], EC[:, s_slot, 128 * ch : 128 * ch + 128],
                        ID[:, :],
                    )
                    ct = ctp.tile([128, 128], BF16, tag="ct")
                    nc.scalar.copy(ct, cpf[:, 0:128])
                    ctb = bass.AP(
                        tensor=ct.tensor, offset=ct.offset,
                        ap=[ct.ap[0], [0, DIN], [64, 2], [1, B]],
                    )
                    xkb = bass.AP(
                        tensor=XKT.tensor,
                        offset=XKT.offset + ch * (DIN * B),
                        ap=[XKT.ap[0], [B, DIN], [0, 2], [1, B]],
                    )
                    yk = ykp.tile([128, DIN, 2, B], BF16, tag="yk")
                    nc.vector.tensor_tensor(yk, ctb, xkb, op=mybir.AluOpType.mult)
                    return lambda t, k: yk[:, k, t, :]

                s_phase(1, coeff2)
            _squash_padded(nc, sqp, SGLB, VV, BF16)
            v2_prep()

            # ---------------- iteration 3 ----------------
            if SKIP_U:
                nc.vector.memset(EC, 1.0)
            else:
                u_phase(2)
            softmax(EC, E2)

            with (
                tc.tile_pool(name="psc3", bufs=3, space="PSUM") as psc,
                tc.tile_pool(name="ct3", bufs=3) as ctp,
                tc.tile_pool(name="yk3", bufs=3) as ykp,
            ):
                def coeff3(s_slot, ch):
                    if SKIP_YK:
                        return lambda t, k: XKT[:, ch, k, :]
                    cpf = psc.tile([128, 1024], BF16, tag="cp",
                                   name=f"cp3_{s_slot}_{ch}")
                    nc.tensor.transpose(
                        cpf[:, 0:128], E2[:, s_slot, 128 * ch : 128 * ch + 128],
                        ID[:, :],
                    )
                    ct = ctp.tile([128, 128], BF16, tag="ct")
                    nc.scalar.copy(ct, cpf[:, 0:128])
                    ctb = bass.AP(
                        tensor=ct.tensor, offset=ct.offset,
                        ap=[ct.ap[0], [0, DIN], [64, 2], [1, B]],
                    )
                    xkb = bass.AP(
                        tensor=XKT.tensor,
                        offset=XKT.offset + ch * (DIN * B),
                        ap=[XKT.ap[0], [B, DIN], [0, 2], [1, B]],
                    )
                    yk = ykp.tile([128, DIN, 2, B], BF16, tag="yk")
                    nc.vector.tensor_tensor(yk, ctb, xkb, op=mybir.AluOpType.mult)
                    return lambda t, k: yk[:, k, t, :]

                s_phase(2, coeff3)
            _squash_packed(nc, sqp, SGLB, V3)

            # ---------------- output ----------------
            for t in range(2):
                vsl = V3[64 * t : 64 * t + 64]
                src = bass.AP(
                    tensor=vsl.tensor, offset=vsl.offset,
                    ap=[vsl.ap[0], [4 * DOUT, 8], [DOUT, 4], [1, DOUT]],
                )
                obase = out[:, :, :]
                dst = bass.AP(
                    tensor=obase.tensor,
                    offset=obase.offset + (4 * t) * DOUT,
                    ap=[[O * DOUT, B], [8 * DOUT, 8], [DOUT, 4], [1, DOUT]],
                )
                nc.sync.dma_start(out=dst, in_=src)
    nc.finalize()
    return nc


def _pack_inputs(x, weight):
    """Host-side packing of per-core shards (numpy, bf16)."""
    bfd = ml_dtypes.bfloat16
    # W2[c][32j+16u+d, T, t, i*8+k] = weight[8T+4t+j, c*256+i, d, k], u=0
    wv = weight.reshape(8, 2, 4, NCORES, IL, DOUT, DIN)  # T,t,j,c,i,d,k
    w2 = np.zeros((NCORES, 4, 2, DOUT, 8, 2, IL * DIN), np.float32)
    w2[:, :, 0] = (
        wv.transpose(3, 2, 5, 0, 1, 6, 4).reshape(NCORES, 4, DOUT, 8, 2, IL * DIN)
    )
    w2 = w2.reshape(NCORES, 128, 8, 2, IL * DIN).astype(bfd)
    # Wk[c][ip, ch, o, k, d] = weight[o, c*256+ch*128+ip, d, k]
    wv2 = weight.reshape(O, NCORES, ICH, 128, DOUT, DIN)
    wkp = wv2.transpose(1, 3, 2, 0, 5, 4).astype(bfd)  # c,ip,ch,o,k,d
    # x2d[c][p, i*8+k] = x[p%64, c*256+i, k]
    xv = x.reshape(B, NCORES, IL, DIN)
    x2 = xv.transpose(1, 0, 3, 2).reshape(NCORES, B, IL * DIN)
    x2d = np.concatenate([x2, x2], axis=1).astype(bfd)
    # xkT[c][ip, ch, k, b] = x[b, c*256+ch*128+ip, k]
    xv2 = x.reshape(B, NCORES, ICH, 128, DIN)
    xkt = xv2.transpose(1, 3, 2, 4, 0).astype(bfd)
    # Wf[c][r, cc, o*16+d] = weight[o, i(f), d, k(f)], f = cc*128+r = i*8+k
    wv3 = weight.transpose(1, 3, 0, 2).reshape(NCORES, 16, 128, O * DOUT)
    wfp = wv3.transpose(0, 2, 1, 3).astype(bfd)
    # xik[c][r, cc, b] = x[b, i(f), k(f)]
    xv3 = x.transpose(1, 2, 0).reshape(NCORES, 16, 128, B)
    xikp = xv3.transpose(0, 2, 1, 3).astype(bfd)
    idn = np.eye(128, dtype=np.float32).astype(bfd)
    return w2, wkp, wfp, xikp, x2d, xkt, idn


_CACHE = {}


def _make_runner(nc, key, nruns=1):
    """Cached PJRT runner (same chaining scheme as the baseline)."""
    ck = (key, nruns)
    if ck in _CACHE:
        return _CACHE[ck]
    import jax
    from jax.sharding import Mesh, PartitionSpec, NamedSharding
    from jax.experimental.shard_map import shard_map
    from concourse import bass2jax as b2j

    b2j.install_neuronx_cc_hook()
    partition_name = nc.partition_id_tensor.name if nc.partition_id_tensor else None
    in_names, out_names, out_avals, zero_outs = [], [], [], []
    for alloc in nc.m.functions[0].allocations:
        if not isinstance(alloc, mybir.MemoryLocationSet):
            continue
        name = alloc.memorylocations[0].name
        if alloc.kind == "ExternalInput":
            if name != partition_name:
                in_names.append(name)
        elif alloc.kind == "ExternalOutput":
            out_names.append(name)
            shape = tuple(alloc.tensor_shape)
            dtype = mybir.dt.np(alloc.dtype)
            out_avals.append(jax.core.ShapedArray(shape, dtype))
            zero_outs.append(np.zeros(shape, dtype))
    assert len(out_names) == 1
    n_params = len(in_names)
    all_names = list(in_names) + list(out_names)
    if partition_name is not None:
        all_names.append(partition_name)
    donate = (n_params,)

    def _body(*args):
        params = list(args[:n_params])
        z = args[n_params]
        for _ in range(nruns):
            operands = params + [z]
            if partition_name is not None:
                operands.append(b2j.partition_id_tensor())
            (z,) = b2j._bass_exec_p.bind(
                *operands,
                out_avals=tuple(out_avals),
                in_names=tuple(all_names),
                out_names=tuple(out_names),
                lowering_input_output_aliases=(),
                sim_require_finite=True,
                sim_require_nnan=True,
                nc=nc,
            )
        return (z,)

    devices = jax.devices()[:NCORES]
    mesh = Mesh(np.asarray(devices), ("core",))
    in_specs = (PartitionSpec("core"),) * (n_params + 1)
    out_specs = (PartitionSpec("core"),)
    sharded = jax.jit(
        shard_map(_body, mesh=mesh, in_specs=in_specs, out_specs=out_specs,
                  check_rep=False),
        donate_argnums=donate, keep_unused=True,
    )
    sharding = NamedSharding(mesh, PartitionSpec("core"))

    def put_inputs(in_maps):
        return [
            jax.device_put(
                np.concatenate(
                    [np.asarray(in_maps[c][nm]) for c in range(NCORES)], axis=0
                ),
                sharding,
            )
            for nm in in_names
        ]

    def run(dev_in):
        z = np.zeros(
            (NCORES * zero_outs[0].shape[0], *zero_outs[0].shape[1:]),
            zero_outs[0].dtype,
        )
        (o,) = sharded(*dev_in, z)
        o = jax.block_until_ready(o)
        return np.asarray(o).reshape(NCORES, *out_avals[0].shape)

    r = (put_inputs, run)
    _CACHE[ck] = r
    _CACHE[f"sharded_{key}"] = sharded
    _CACHE[f"zshape_{key}"] = (
        NCORES * zero_outs[0].shape[0], *zero_outs[0].shape[1:]
    )
    return r


def _in_maps(x, weight):
    w2, wkp, wfp, xikp, x2d, xkt, idn = _pack_inputs(
        np.asarray(x, dtype=np.float32), np.asarray(weight, dtype=np.float32)
    )
    return [
        {"w2": w2[c], "wk": wkp[c], "wf": wfp[c], "xik": xikp[c],
         "x2d": x2d[c], "xkt": xkt[c], "idn": idn}
        for c in range(NCORES)
    ]


def kernel(x, weight):
    if "nc" not in _CACHE:
        _CACHE["nc"] = build()
    put, run = _make_runner(_CACHE["nc"], "main", 1)
    outs = run(put(_in_maps(x, weight)))
    return np.asarray(outs[0], dtype=np.float32)


def measure(x, weight, nqueue=32, reps=3):
    """Per-execution device time via chained donated-buffer executions."""
    import time
    import jax
    if "nc" not in _CACHE:
        _CACHE["nc"] = build()
    nc = _CACHE["nc"]
    maps = _in_maps(x, weight)
    put1, run1 = _make_runner(nc, "main", 1)
    dev = put1(maps)
    sharded = _CACHE["sharded_main"]
    zshape = _CACHE["zshape_main"]

    def chain(k):
        z = np.zeros(zshape, np.float32)
        for _ in range(k):
            (z,) = sharded(*dev, z)
        return z

    jax.block_until_ready(chain(2))
    t1s, tks = [], []
    for _ in range(reps):
        t0 = time.perf_counter_ns()
        jax.block_until_ready(chain(1))
        t1s.append(time.perf_counter_ns() - t0)
        t0 = time.perf_counter_ns()
        jax.block_until_ready(chain(nqueue))
        tks.append(time.perf_counter_ns() - t0)
    per_exec = (min(tks) - min(t1s)) / (nqueue - 1)
    return int(per_exec), min(t1s), min(tks)


# revision 8
# speedup vs baseline: 1.9578x; 1.9578x over previous
"""DenseCapsule routing kernel for 8 Trainium2 NeuronCores — factorized.

Problem: x [B=64, I=2048, Din=8], weight [O=64, I=2048, Dout=16, Din=8]
  x_hat = einsum('oidk,bik->boid', w, x); 3 rounds of dynamic routing
  (softmax over O, weighted i-sum, squash, agreement update); out [B, O, Dout].

Strategy: shard I across the 8 cores (IL=256 each). x_hat is NEVER
materialized. Both routing contractions factor through W:
  s[b,o,d]  = sum_{i,k} W[o,i,d,k] * (c[b,o,i] x[b,i,k])   (PE, contract i)
  db[b,o,i] = sum_k x[b,i,k] * u[b,o,i,k],
  u[b,o,i,k]= sum_d W[o,i,d,k] v[b,o,d]                    (PE, contract d)
The only HBM traffic after setup is one [128,1024] f32 AllReduce of the
s-partials per routing iteration.

Index conventions (per core):
  o = 8T + 4t + j,  T in [0,8), t in {0,1}, j in [0,4)
  softmax slot s = 4T + j in [0,32); SBUF partition p = 64t + b%64
  padded (j,u,d) free index 32j + 16u + d (u=1 slots are zero padding)
"""

import sys

sys.path.insert(0, "/opt/trn_rl_repo")

import numpy as np
import ml_dtypes

import concourse.bass as bass
import concourse.tile as tile
from concourse import bacc, mybir
from concourse.bass_utils import run_bass_kernel_spmd

F32 = mybir.dt.float32
BF16 = mybir.dt.bfloat16

B, I, DIN, O, DOUT = 64, 2048, 8, 64, 16
NCORES = 8
IL = I // NCORES          # 256 i's per core
ICH = IL // 128           # 2 i-chunks of 128
EPS = 1e-8
PAD = 128                 # padded (j,u,d) block
SKIP_U = False            # timing variant: skip u_phases
SKIP_YK = False           # timing variant: s-MM rhs = XKT (no cT/yk)
SKIP_AR = False           # timing variant: local DMA instead of AllReduce
SKIP_SMM = False          # timing variant: skip s-MM/evict/transpose in s_phase
SFREE = 512               # packed (T,j,d) free elems of the AR payload


def _squash_padded(nc, pool, sgl, vout, vdtype):
    """vout[p, T, 32j+d] = squash(sgl) over d; padded layout, u=0 slots only.

    sgl: [128, 8, 128] f32 SBUF; vout [128, 8, 128] (u=1 slots untouched).
    """
    sv = sgl[:, :, :, :]  # packed [p, T, j, d]
    n2 = pool.tile([128, 8, 4], F32, tag="n2")
    sq = pool.tile([128, 8, 4, DOUT], F32, tag="sq")
    nc.vector.tensor_tensor(sq, sv, sv, op=mybir.AluOpType.mult)
    nc.vector.tensor_reduce(
        n2, sq, axis=mybir.AxisListType.X, op=mybir.AluOpType.add,
    )
    np1 = pool.tile([128, 8, 4], F32, tag="np1")
    nc.vector.tensor_scalar_add(np1, n2, 1.0)
    r1 = pool.tile([128, 8, 4], F32, tag="r1")
    nc.vector.reciprocal(r1, np1)
    nrm = pool.tile([128, 8, 4], F32, tag="nrm")
    nc.scalar.activation(nrm, n2, mybir.ActivationFunctionType.Sqrt)
    nre = pool.tile([128, 8, 4], F32, tag="nre")
    nc.vector.tensor_scalar_add(nre, nrm, EPS)
    r2 = pool.tile([128, 8, 4], F32, tag="r2")
    nc.vector.reciprocal(r2, nre)
    sc = pool.tile([128, 8, 4], F32, tag="sc")
    nc.vector.tensor_tensor(sc, n2, r1, op=mybir.AluOpType.mult)
    sc2 = pool.tile([128, 8, 4], F32, tag="sc2")
    nc.vector.tensor_tensor(sc2, sc, r2, op=mybir.AluOpType.mult)
    scb = bass.AP(
        tensor=sc2.tensor, offset=sc2.offset,
        ap=[sc2.ap[0], [4, 8], [1, 4], [0, DOUT]],
    )
    vv = bass.AP(
        tensor=vout.tensor, offset=vout.offset,
        ap=[vout.ap[0], [PAD, 8], [32, 4], [1, DOUT]],
    )
    nc.vector.tensor_tensor(vv, sv, scb, op=mybir.AluOpType.mult)


def _squash_packed(nc, pool, sgl, vout):
    """vout (packed [128, 8, 4, 16] f32) = squash(sgl bf16) over d."""
    sv = sgl[:, :, :, :]
    n2 = pool.tile([128, 8, 4], F32, tag="n2")
    sq = pool.tile([128, 8, 4, DOUT], F32, tag="sq")
    nc.vector.tensor_tensor(sq, sv, sv, op=mybir.AluOpType.mult)
    nc.vector.tensor_reduce(
        n2, sq, axis=mybir.AxisListType.X, op=mybir.AluOpType.add,
    )
    np1 = pool.tile([128, 8, 4], F32, tag="np1")
    nc.vector.tensor_scalar_add(np1, n2, 1.0)
    r1 = pool.tile([128, 8, 4], F32, tag="r1")
    nc.vector.reciprocal(r1, np1)
    nrm = pool.tile([128, 8, 4], F32, tag="nrm")
    nc.scalar.activation(nrm, n2, mybir.ActivationFunctionType.Sqrt)
    nre = pool.tile([128, 8, 4], F32, tag="nre")
    nc.vector.tensor_scalar_add(nre, nrm, EPS)
    r2 = pool.tile([128, 8, 4], F32, tag="r2")
    nc.vector.reciprocal(r2, nre)
    sc = pool.tile([128, 8, 4], F32, tag="sc")
    nc.vector.tensor_tensor(sc, n2, r1, op=mybir.AluOpType.mult)
    sc2 = pool.tile([128, 8, 4], F32, tag="sc2")
    nc.vector.tensor_tensor(sc2, sc, r2, op=mybir.AluOpType.mult)
    scb = bass.AP(
        tensor=sc2.tensor, offset=sc2.offset,
        ap=[sc2.ap[0], [4, 8], [1, 4], [0, DOUT]],
    )
    nc.vector.tensor_tensor(vout, sv, scb, op=mybir.AluOpType.mult)


def build():
    nc = bacc.Bacc()
    w2 = nc.declare_dram_parameter("w2", [128, 8, 2, IL * DIN], BF16, isOutput=False)
    wk = nc.declare_dram_parameter("wk", [128, ICH, O, DIN, DOUT], BF16, isOutput=False)
    wf = nc.declare_dram_parameter("wf", [128, 16, O * DOUT], BF16, isOutput=False)
    xik = nc.declare_dram_parameter("xik", [128, 16, B], BF16, isOutput=False)
    x2d = nc.declare_dram_parameter("x2d", [128, IL * DIN], BF16, isOutput=False)
    xkt = nc.declare_dram_parameter("xkt", [128, ICH, DIN, B], BF16, isOutput=False)
    idn = nc.declare_dram_parameter("idn", [128, 128], BF16, isOutput=False)
    out = nc.declare_dram_parameter("out", [B, O, DOUT], F32, isOutput=True)

    groups = [list(range(NCORES))]

    with tile.TileContext(nc) as tc:
        with (
            tc.tile_pool(name="dram", bufs=1, space="DRAM") as dram,
            tc.tile_pool(name="consts", bufs=1) as consts,
            tc.tile_pool(name="persist", bufs=1) as persist,
            tc.tile_pool(name="sqp", bufs=1) as sqp,
        ):
            sp = [dram.tile([128, SFREE], F32, name=f"sp{q}") for q in range(3)]
            sr = [
                dram.tile([128, SFREE], F32, addr_space="Shared", name=f"sr{q}")
                for q in range(3)
            ]

            WF = consts.tile([128, 16, O * DOUT], BF16)
            nc.sync.dma_start(out=WF, in_=wf[:, :, :])
            XIK = consts.tile([128, 16, B], BF16)
            nc.sync.dma_start(out=XIK, in_=xik[:, :, :])
            ID = consts.tile([128, 128], BF16)
            nc.sync.dma_start(out=ID, in_=idn[:, :])
            W2 = consts.tile([128, 8, 2, IL * DIN], BF16)
            X2D = consts.tile([128, IL * DIN], BF16)
            XKT = consts.tile([128, ICH, DIN, B], BF16)
            WK = consts.tile([128, ICH, O, DIN, DOUT], BF16)

            sE = persist.tile([128, 16, B], BF16)     # evicted s-MM slices
            nc.vector.memset(sE, 0.0)
            SGLS = persist.tile([128, 8, 4, DOUT], F32)   # packed s partials
            SGLB = persist.tile([128, 8, 4, DOUT], F32)   # all-reduced s
            V3 = persist.tile([128, 8, 4, DOUT], F32)     # final v (f32)
            VV = persist.tile([128, 8, PAD], BF16)    # v (padded, bf16)
            nc.vector.memset(VV, 0.0)
            V2 = persist.tile([128, 8, 2, B], BF16)   # transposed v for u-MM
            E2 = persist.tile([128, 32, IL], BF16)    # exp(db2)
            EC = persist.tile([128, 32, IL], BF16)    # c2, then e3
            SMS = persist.tile([128, IL], F32)        # softmax sum (both halves)
            SMR = persist.tile([128, IL], BF16)       # softmax recip (bcast)

            def do_ar(q):
                if SKIP_AR:
                    nc.sync.dma_start(out=sr[q][:], in_=sp[q][:])
                else:
                    nc.gpsimd.collective_compute(
                        "AllReduce", mybir.AluOpType.add, replica_groups=groups,
                        ins=[sp[q][:]], outs=[sr[q][:]],
                    )

            def s_phase(q, coeff):
                """s-MMs for all o using rhs slices from coeff(s_slot, ch, t, k)
                -> AllReduce q -> SGL."""
                with (
                    tc.tile_pool(name=f"ps_s{q}", bufs=2, space="PSUM") as pss,
                    tc.tile_pool(name=f"ps_t{q}", bufs=2, space="PSUM") as pst,
                    tc.tile_pool(name=f"ev{q}", bufs=2) as ev,
                ):
                    for T in range(8):
                        if SKIP_SMM:
                            for j in range(4):
                                for ch in range(ICH):
                                    coeff(4 * T + j, ch)
                            continue
                        s8f = [
                            pss.tile([128, 512], F32, tag="s8",
                                     name=f"s8_{q}_{T}_{t}")
                            for t in range(2)
                        ]
                        s8 = [sf[:, 0:B] for sf in s8f]
                        for j in range(4):
                            s_slot = 4 * T + j
                            for ch in range(ICH):
                                rhs_td = coeff(s_slot, ch)
                                for t in range(2):
                                    o = 8 * T + 4 * t + j
                                    for k in range(DIN):
                                        nc.tensor.matmul(
                                            s8f[t][32 * j : 32 * j + 16, 0:B],
                                            WK[:, ch, o, k, :],
                                            rhs_td(t, k),
                                            start=(ch == 0 and k == 0),
                                            stop=(ch == ICH - 1 and k == DIN - 1),
                                            tile_position=(0, 32 * j),
                                        )
                        # evict the 4 real 16-row slices of each half into sE
                        for t in range(2):
                            G = 2 * T + t
                            for j in range(4):
                                nc.scalar.copy(
                                    sE[32 * j : 32 * j + 16, G, :],
                                    s8[t][32 * j : 32 * j + 16, :],
                                )
                        # transpose the (t-pair) into [p=(t,b), (j,u,d)]
                        tpf = pst.tile([128, 1024], BF16, tag="tp",
                                       name=f"tp{q}_{T}")
                        nc.tensor.transpose(
                            tpf[:, 0:128], sE[:, 2 * T : 2 * T + 2, :], ID[:, :]
                        )
                        tpp = bass.AP(  # select u=0: [p, j, d]
                            tensor=tpf.tensor, offset=tpf.offset,
                            ap=[tpf.ap[0], [32, 4], [1, DOUT]],
                        )
                        nc.scalar.copy(SGLS[:, T, :, :], tpp)
                    nc.sync.dma_start(
                        out=sp[q],
                        in_=SGLS.rearrange("p a b c -> p (a b c)"),
                    )
                do_ar(q)
                nc.sync.dma_start(
                    out=SGLB.rearrange("p a b c -> p (a b c)"), in_=sr[q][:]
                )

            def v2_prep():
                """V2[32j+16u+d, T, t, b] = VV transposed per (T, t)."""
                with (
                    tc.tile_pool(name="psv", bufs=2, space="PSUM") as psv,
                ):
                    for T in range(8):
                        for t in range(2):
                            vpf = psv.tile([128, 1024], BF16, tag="vp")
                            nc.tensor.transpose(
                                vpf[:, 0:B],
                                VV[64 * t : 64 * t + 64, T, :],
                                ID[64 * t : 64 * t + 64, 64 * t : 64 * t + 64],
                            )
                            nc.scalar.copy(V2[:, T, t, :], vpf[:, 0:B])

            def u_phase(it):
                """u-MMs + db + exp into E2 (it=1) or EC=E2*exp(db3) (it=2).

                W2/X2D free index is k-major: k*IL + i. The k-sum is a
                unit-stride TT-add tree (2x mode; tensor_reduce is 1x-only).
                """
                with (
                    tc.tile_pool(name=f"psu{it}", bufs=2, space="PSUM") as psu,
                    tc.tile_pool(name=f"ub{it}", bufs=2) as ubp,
                    tc.tile_pool(name=f"tt{it}", bufs=2) as ttp,
                    tc.tile_pool(name=f"db{it}", bufs=3) as dbp,
                ):
                    for T in range(8):
                        for j in range(4):
                            s_slot = 4 * T + j
                            pu = psu.tile(
                                [128, 2048], F32, tag="pu",
                                name=f"pu{it}_{T}_{j}",
                            )
                            for t in range(2):
                                for h in range(4):
                                    nc.tensor.matmul(
                                        pu[64 * t : 64 * t + 64,
                                           512 * h : 512 * h + 512],
                                        V2[32 * j : 32 * j + 16, T, t, :],
                                        W2[32 * j : 32 * j + 16, T, t,
                                           512 * h : 512 * h + 512],
                                        start=True, stop=True,
                                        tile_position=(32 * j, 64 * t),
                                    )
                            ub = ubp.tile([128, 2048], BF16, tag="ub")
                            nc.scalar.copy(ub, pu)
                            tt = ttp.tile([128, DIN, IL], BF16, tag="tt")
                            nc.vector.tensor_tensor(
                                tt,
                                ub.rearrange("p (k i) -> p k i", k=DIN),
                                X2D.rearrange("p (k i) -> p k i", k=DIN),
                                op=mybir.AluOpType.mult,
                            )
                            nc.vector.tensor_tensor(
                                tt[:, 0:4, :], tt[:, 0:4, :], tt[:, 4:8, :],
                                op=mybir.AluOpType.add,
                            )
                            nc.vector.tensor_tensor(
                                tt[:, 0:2, :], tt[:, 0:2, :], tt[:, 2:4, :],
                                op=mybir.AluOpType.add,
                            )
                            db = dbp.tile([128, IL], BF16, tag="db")
                            nc.vector.tensor_tensor(
                                db, tt[:, 0, :], tt[:, 1, :],
                                op=mybir.AluOpType.add,
                            )
                            if it == 1:
                                nc.scalar.activation(
                                    E2[:, s_slot, :], db,
                                    mybir.ActivationFunctionType.Exp,
                                )
                            else:
                                ex = dbp.tile([128, IL], BF16, tag="ex")
                                nc.scalar.activation(
                                    ex, db, mybir.ActivationFunctionType.Exp,
                                )
                                nc.vector.tensor_tensor(
                                    EC[:, s_slot, :],
                                    E2[:, s_slot, :], ex,
                                    op=mybir.AluOpType.mult,
                                )

            def softmax(e_buf, c_buf):
                """c = e / sum_o e ; sum over 32 slots on both halves."""
                with tc.tile_pool(name="smx", bufs=1) as smx:
                    s16 = smx.tile([128, 16, IL], BF16, tag="s16")
                    nc.vector.tensor_tensor(
                        s16, e_buf[:, 0:16, :], e_buf[:, 16:32, :],
                        op=mybir.AluOpType.add,
                    )
                    nc.vector.tensor_tensor(
                        s16[:, 0:8, :], s16[:, 0:8, :], s16[:, 8:16, :],
                        op=mybir.AluOpType.add,
                    )
                    nc.vector.tensor_tensor(
                        s16[:, 0:4, :], s16[:, 0:4, :], s16[:, 4:8, :],
                        op=mybir.AluOpType.add,
                    )
                    nc.vector.tensor_tensor(
                        s16[:, 0:2, :], s16[:, 0:2, :], s16[:, 2:4, :],
                        op=mybir.AluOpType.add,
                    )
                    with nc.allow_low_precision("softmax sum in f32 out"):
                        nc.vector.tensor_tensor(
                            SMS, s16[:, 0, :], s16[:, 1, :],
                            op=mybir.AluOpType.add,
                        )
                    half = smx.tile([64, IL], F32, tag="half")
                    nc.sync.dma_start(out=half, in_=SMS[64:128, :])
                    tot = smx.tile([64, IL], F32, tag="tot")
                    nc.vector.tensor_tensor(
                        tot, SMS[0:64, :], half, op=mybir.AluOpType.add,
                    )
                    rec = smx.tile([64, IL], F32, tag="rec")
                    nc.vector.reciprocal(rec, tot)
                    nc.vector.tensor_copy(SMR[0:64, :], rec)
                    nc.sync.dma_start(out=SMR[64:128, :], in_=SMR[0:64, :])
                    rb = bass.AP(
                        tensor=SMR.tensor, offset=SMR.offset,
                        ap=[SMR.ap[0], [0, 32], [1, IL]],
                    )
                    nc.vector.tensor_tensor(
                        c_buf, e_buf, rb, op=mybir.AluOpType.mult,
                    )

            # ---------------- iteration 1: uniform c, one big matmul ----
            with (
                tc.tile_pool(name="ps1", bufs=1, space="PSUM") as ps1,
                tc.tile_pool(name="s1e", bufs=1) as s1e,
            ):
                p1 = ps1.tile([B, O * DOUT], F32, tag="p1")
                for c in range(16):
                    for h in range(2):
                        nc.tensor.matmul(
                            p1[:, 512 * h : 512 * h + 512],
                            XIK[:, c, :],
                            WF[:, c, 512 * h : 512 * h + 512],
                            start=(c == 0), stop=(c == 15),
                        )
                # evict * (1/O) with on-the-fly reorder to packed layout:
                # s1p[b, t, 64T+16j+d] = p1[b, (8T+4t+j)*16+d] / O
                s1p = s1e.tile([B, 2, SFREE], F32, tag="s1p")
                for t in range(2):
                    pin = bass.AP(
                        tensor=p1.tensor,
                        offset=p1.offset + (4 * t) * DOUT,
                        ap=[p1.ap[0], [8 * DOUT, 8], [DOUT, 4], [1, DOUT]],
                    )
                    nc.scalar.mul(s1p[:, t, :].rearrange(
                        "b (T jd) -> b T jd", T=8), pin, 1.0 / O)
                for t in range(2):
                    dst = bass.AP(
                        tensor=sp[0].tensor,
                        offset=sp[0].offset + 64 * t * SFREE,
                        ap=[[SFREE, B], [1, SFREE]],
                    )
                    nc.sync.dma_start(out=dst, in_=s1p[:, t, :])
            nc.sync.dma_start(out=W2, in_=w2[:, :, :, :])
            nc.sync.dma_start(out=X2D, in_=x2d[:, :])
            nc.sync.dma_start(out=XKT, in_=xkt[:, :, :, :])
            nc.sync.dma_start(out=WK, in_=wk[:, :, :, :, :])
            do_ar(0)
            nc.sync.dma_start(
                out=SGLB.rearrange("p a b c -> p (a b c)"), in_=sr[0][:]
            )
            _squash_padded(nc, sqp, SGLB, VV, BF16)
            v2_prep()

            # ---------------- iteration 2 ----------------
            if SKIP_U:
                nc.vector.memset(E2, 1.0)
            else:
                u_phase(1)
            softmax(E2, EC)

            with (
                tc.tile_pool(name="psc2", bufs=3, space="PSUM") as psc,
                tc.tile_pool(name="ct2", bufs=3) as ctp,
                tc.tile_pool(name="yk2", bufs=3) as ykp,
            ):
                def coeff2(s_slot, ch):
                    if SKIP_YK:
                        return lambda t, k: XKT[:, ch, k, :]
                    cpf = psc.tile([128, 1024], BF16, tag="cp",
                                   name=f"cp2_{s_slot}_{ch}")
                    nc.tensor.transpose(
                        cpf[:, 0:128], EC[:, s_slot, 128 * ch : 128 * ch + 128],
                        ID[:, :],
                    )
                    ct = ctp.tile([128, 128], BF16, tag="ct")
                    nc.scalar.copy(ct, cpf[:, 0:128])
                    ctb = bass.AP(
                        tensor=ct.tensor, offset=ct.offset,
                        ap=[ct.ap[0], [0, DIN], [64, 2], [1, B]],
                    )
                    xkb = bass.AP(
                        tensor=XKT.tensor,
                        offset=XKT.offset + ch * (DIN * B),
                        ap=[XKT.ap[0], [B, DIN], [0, 2], [1, B]],
                    )
                    yk = ykp.tile([128, DIN, 2, B], BF16, tag="yk")
                    nc.vector.tensor_tensor(yk, ctb, xkb, op=mybir.AluOpType.mult)
                    return lambda t, k: yk[:, k, t, :]

                s_phase(1, coeff2)
            _squash_padded(nc, sqp, SGLB, VV, BF16)
            v2_prep()

            # ---------------- iteration 3 ----------------
            if SKIP_U:
                nc.vector.memset(EC, 1.0)
            else:
                u_phase(2)
            softmax(EC, E2)

            with (
                tc.tile_pool(name="psc3", bufs=3, space="PSUM") as psc,
                tc.tile_pool(name="ct3", bufs=3) as ctp,
                tc.tile_pool(name="yk3", bufs=3) as ykp,
            ):
                def coeff3(s_slot, ch):
                    if SKIP_YK:
                        return lambda t, k: XKT[:, ch, k, :]
                    cpf = psc.tile([128, 1024], BF16, tag="cp",
                                   name=f"cp3_{s_slot}_{ch}")
                    nc.tensor.transpose(
                        cpf[:, 0:128], E2[:, s_slot, 128 * ch : 128 * ch + 128],
                        ID[:, :],
                    )
                    ct = ctp.tile([128, 128], BF16, tag="ct")
                    nc.scalar.copy(ct, cpf[:, 0:128])
                    ctb = bass.AP(
                        tensor=ct.tensor, offset=ct.offset,
                        ap=[ct.ap[0], [0, DIN], [64, 2], [1, B]],
                    )
                    xkb = bass.AP(
                        tensor=XKT.tensor,
                        offset=XKT.offset + ch * (DIN * B),
                        ap=[XKT.ap[0], [B, DIN], [0, 2], [1, B]],
                    )
                    yk = ykp.tile([128, DIN, 2, B], BF16, tag="yk")
                    nc.vector.tensor_tensor(yk, ctb, xkb, op=mybir.AluOpType.mult)
                    return lambda t, k: yk[:, k, t, :]

                s_phase(2, coeff3)
            _squash_packed(nc, sqp, SGLB, V3)

            # ---------------- output ----------------
            for t in range(2):
                vsl = V3[64 * t : 64 * t + 64]
                src = bass.AP(
                    tensor=vsl.tensor, offset=vsl.offset,
                    ap=[vsl.ap[0], [4 * DOUT, 8], [DOUT, 4], [1, DOUT]],
                )
                obase = out[:, :, :]
                dst = bass.AP(
                    tensor=obase.tensor,
                    offset=obase.offset + (4 * t) * DOUT,
                    ap=[[O * DOUT, B], [8 * DOUT, 8], [DOUT, 4], [1, DOUT]],
                )
                nc.sync.dma_start(out=dst, in_=src)
    nc.finalize()
    return nc


def _pack_inputs(x, weight):
    """Host-side packing of per-core shards (numpy, bf16)."""
    bfd = ml_dtypes.bfloat16
    # W2[c][32j+16u+d, T, t, i*8+k] = weight[8T+4t+j, c*256+i, d, k], u=0
    wv = weight.reshape(8, 2, 4, NCORES, IL, DOUT, DIN)  # T,t,j,c,i,d,k
    w2 = np.zeros((NCORES, 4, 2, DOUT, 8, 2, IL * DIN), np.float32)
    w2[:, :, 0] = (
        wv.transpose(3, 2, 5, 0, 1, 6, 4).reshape(NCORES, 4, DOUT, 8, 2, IL * DIN)
    )
    w2 = w2.reshape(NCORES, 128, 8, 2, IL * DIN).astype(bfd)
    # Wk[c][ip, ch, o, k, d] = weight[o, c*256+ch*128+ip, d, k]
    wv2 = weight.reshape(O, NCORES, ICH, 128, DOUT, DIN)
    wkp = wv2.transpose(1, 3, 2, 0, 5, 4).astype(bfd)  # c,ip,ch,o,k,d
    # x2d[c][p, i*8+k] = x[p%64, c*256+i, k]
    xv = x.reshape(B, NCORES, IL, DIN)
    x2 = xv.transpose(1, 0, 3, 2).reshape(NCORES, B, IL * DIN)
    x2d = np.concatenate([x2, x2], axis=1).astype(bfd)
    # xkT[c][ip, ch, k, b] = x[b, c*256+ch*128+ip, k]
    xv2 = x.reshape(B, NCORES, ICH, 128, DIN)
    xkt = xv2.transpose(1, 3, 2, 4, 0).astype(bfd)
    # Wf[c][r, cc, o*16+d] = weight[o, i(f), d, k(f)], f = cc*128+r = i*8+k
    wv3 = weight.transpose(1, 3, 0, 2).reshape(NCORES, 16, 128, O * DOUT)
    wfp = wv3.transpose(0, 2, 1, 3).astype(bfd)
    # xik[c][r, cc, b] = x[b, i(f), k(f)]
    xv3 = x.transpose(1, 2, 0).reshape(NCORES, 16, 128, B)
    xikp = xv3.transpose(0, 2, 1, 3).astype(bfd)
    idn = np.eye(128, dtype=np.float32).astype(bfd)
    return w2, wkp, wfp, xikp, x2d, xkt, idn


_CACHE = {}


def _make_runner(nc, key, nruns=1):
    """Cached PJRT runner (same chaining scheme as the baseline)."""
    ck = (key, nruns)
    if ck in _CACHE:
        return _CACHE[ck]
    import jax
    from jax.sharding import Mesh, PartitionSpec, NamedSharding
    from jax.experimental.shard_map import shard_map
    from concourse import bass2jax as b2j

    b2j.install_neuronx_cc_hook()
    partition_name = nc.partition_id_tensor.name if nc.partition_id_tensor else None
    in_names, out_names, out_avals, zero_outs = [], [], [], []
    for alloc in nc.m.functions[0].allocations:
        if not isinstance(alloc, mybir.MemoryLocationSet):
            continue
        name = alloc.memorylocations[0].name
        if alloc.kind == "ExternalInput":
            if name != partition_name:
                in_names.append(name)
        elif alloc.kind == "ExternalOutput":
            out_names.append(name)
            shape = tuple(alloc.tensor_shape)
            dtype = mybir.dt.np(alloc.dtype)
            out_avals.append(jax.core.ShapedArray(shape, dtype))
            zero_outs.append(np.zeros(shape, dtype))
    assert len(out_names) == 1
    n_params = len(in_names)
    all_names = list(in_names) + list(out_names)
    if partition_name is not None:
        all_names.append(partition_name)
    donate = (n_params,)

    def _body(*args):
        params = list(args[:n_params])
        z = args[n_params]
        for _ in range(nruns):
            operands = params + [z]
            if partition_name is not None:
                operands.append(b2j.partition_id_tensor())
            (z,) = b2j._bass_exec_p.bind(
                *operands,
                out_avals=tuple(out_avals),
                in_names=tuple(all_names),
                out_names=tuple(out_names),
                lowering_input_output_aliases=(),
                sim_require_finite=True,
                sim_require_nnan=True,
                nc=nc,
            )
        return (z,)

    devices = jax.devices()[:NCORES]
    mesh = Mesh(np.asarray(devices), ("core",))
    in_specs = (PartitionSpec("core"),) * (n_params + 1)
    out_specs = (PartitionSpec("core"),)
    sharded = jax.jit(
        shard_map(_body, mesh=mesh, in_specs=in_specs, out_specs=out_specs,
                  check_rep=False),
        donate_argnums=donate, keep_unused=True,
    )
    sharding = NamedSharding(mesh, PartitionSpec("core"))

    def put_inputs(in_maps):
        return [
            jax.device_put(
                np.concatenate(
                    [np.asarray(in_maps[c][nm]) for c in range(NCORES)], axis=0
                ),
                sharding,
            )
            for nm in in_names
        ]

    def run(dev_in):
        z = np.zeros(
            (NCORES * zero_outs[0].shape[0], *zero_outs[0].shape[1:]),
            zero_outs[0].dtype,
        )
        (o,) = sharded(*dev_in, z)
        o = jax.block_until_ready(o)
        return np.asarray(o).reshape(NCORES, *out_avals[0].shape)

    r = (put_inputs, run)
    _CACHE[ck] = r
    _CACHE[f"sharded_{key}"] = sharded
    _CACHE[f"zshape_{key}"] = (
        NCORES * zero_outs[0].shape[0], *zero_outs[0].shape[1:]
    )
    return r


def _in_maps(x, weight):
    w2, wkp, wfp, xikp, x2d, xkt, idn = _pack_inputs(
        np.asarray(x, dtype=np.float32), np.asarray(weight, dtype=np.float32)
    )
    return [
        {"w2": w2[c], "wk": wkp[c], "wf": wfp[c], "xik": xikp[c],
         "x2d": x2d[c], "xkt": xkt[c], "idn": idn}
        for c in range(NCORES)
    ]


def kernel(x, weight):
    if "nc" not in _CACHE:
        _CACHE["nc"] = build()
    put, run = _make_runner(_CACHE["nc"], "main", 1)
    outs = run(put(_in_maps(x, weight)))
    return np.asarray(outs[0], dtype=np.float32)


def measure(x, weight, nqueue=32, reps=6):
    """Per-execution device time via chained donated-buffer executions."""
    import time
    import jax
    if "nc" not in _CACHE:
        _CACHE["nc"] = build()
    nc = _CACHE["nc"]
    maps = _in_maps(x, weight)
    put1, run1 = _make_runner(nc, "main", 1)
    dev = put1(maps)
    sharded = _CACHE["sharded_main"]
    zshape = _CACHE["zshape_main"]

    def chain(k):
        z = np.zeros(zshape, np.float32)
        for _ in range(k):
            (z,) = sharded(*dev, z)
        return z

    jax.block_until_ready(chain(2))
    t1s, tks = [], []
    for _ in range(reps):
        t0 = time.perf_counter_ns()
        jax.block_until_ready(chain(1))
        t1s.append(time.perf_counter_ns() - t0)
        t0 = time.perf_counter_ns()
        jax.block_until_ready(chain(nqueue))
        tks.append(time.perf_counter_ns() - t0)
    per_exec = (min(tks) - min(t1s)) / (nqueue - 1)
    return int(per_exec), min(t1s), min(tks)
